# revision 14
# baseline (speedup 1.0000x reference)
"""Trainium2 Bass kernel for nn_CognitiveNetwork (16-cell LSTM message-passing net).

Strategy
--------
* Expert-parallel over the C=16 cells: 2 cells per NeuronCore.  All weights
  then fit in SBUF (bf16) and stay resident for the whole scan -- no per-step
  weight traffic (the problem's memory bottleneck).
* Fully "transposed" dataflow: activations live as [H, B] (H on partitions),
  so every bias is a per-partition vector (free via the ACT engine's bias
  operand) and no on-device activation transposes are needed.
* LayerNorm: Sum(p) / Sum(p^2) via ones-vector matmuls on the PE;
  rstd = 1/sqrt(var+eps) via ACT Sqrt + DVE reciprocal_approx_fast;
  the (p - mu) term is folded into the gates matmul as a rank-1 correction
  (lhsT = -rowsum(Wih), rhs = mu*rstd), so normalization costs one DVE
  multiply (p * rstd_broadcast) plus 17 tiny matmuls.
* Per-step cell-mean y: local partial sum, 512KB fp32 AllReduce across the 8
  cores; y feeds the next step's external injection (x_t + 0.3*y).
* Embedding gather + input projection: sharded over cores by timestep (16 t's
  per core), indirect-DMA row gather from the bf16 embedding table, PE
  transpose, projection matmul, then one AllGather of xs^T [T, H, B] bf16.
"""

import os
import sys

sys.path.insert(0, "/opt/trn_rl_repo")

import numpy as np
import ml_dtypes

from concourse import bass, bacc, mybir, tile
from concourse.bass_utils import run_bass_kernel_spmd

BF16 = ml_dtypes.bfloat16

# Problem constants (hardcoded per contract).
V, E, H, C = 50257, 256, 512, 16
B, T = 256, 128
LN_EPS = 1e-5

NCORES = 8
CPC = C // NCORES        # cells per core = 2
HC = H // 128            # h chunks = 4
EC = E // 128            # e chunks = 2
GC = (4 * H) // 128      # gate chunks = 16
TLOC = T // NCORES       # timesteps gathered per core = 16
NGRP = TLOC // 2         # preamble groups per core (2 t's = 512 tokens each)

F32 = mybir.dt.float32
BF = mybir.dt.bfloat16
I32 = mybir.dt.int32
AF = mybir.ActivationFunctionType
ALU = mybir.AluOpType
RG = [list(range(NCORES))]


def _pack_lhsT(w: np.ndarray) -> np.ndarray:
    """Pack [K, M] weight into SBUF lhsT layout [128, (K/128)*(M/128)*128].

    Column block index (k*mc + m)*128 + j holds w[k*128 + p, m*128 + j] at
    partition p.
    """
    K, M = w.shape
    kc, mc = K // 128, M // 128
    return np.ascontiguousarray(
        w.reshape(kc, 128, mc, 128).transpose(1, 0, 2, 3).reshape(128, kc * mc * 128)
    )


def _pack_bias(b: np.ndarray) -> np.ndarray:
    """[n, M] -> [128, n*(M/128)]: column n*idx... (cell-major, chunk-minor)."""
    n, M = b.shape
    mc = M // 128
    return np.ascontiguousarray(
        b.reshape(n, mc, 128).transpose(2, 0, 1).reshape(128, n * mc)
    )


def build_program(t_steps: int = T, dbg: bool = False):
    nc = bacc.Bacc(
        "TRN2",
        target_bir_lowering=False,
        debug=False,
        num_devices=NCORES,
    )
    dbg_d = (
        nc.declare_dram_parameter("dbg", [8, H, B], F32, isOutput=True)
        if dbg else None
    )

    # ---- I/O -------------------------------------------------------------
    emb_d = nc.declare_dram_parameter("emb", [V, E], BF, isOutput=False)
    tok_d = nc.declare_dram_parameter("tok", [NGRP * 4, 128, 1], I32, isOutput=False)
    wproj_d = nc.declare_dram_parameter("wproj", [128, EC * HC * 128], BF, isOutput=False)
    bproj_d = nc.declare_dram_parameter("bproj", [128, HC], F32, isOutput=False)
    wp_d = nc.declare_dram_parameter("wp", [128, CPC * HC * HC * 128], BF, isOutput=False)
    wih_d = nc.declare_dram_parameter("wih", [128, CPC * HC * GC * 128], BF, isOutput=False)
    whh_d = nc.declare_dram_parameter("whh", [128, CPC * HC * GC * 128], BF, isOutput=False)
    wa_d = nc.declare_dram_parameter("wa", [128, CPC * HC * HC * 128], BF, isOutput=False)
    w1n_d = nc.declare_dram_parameter("w1n", [1, CPC * GC * 128], BF, isOutput=False)
    bp_d = nc.declare_dram_parameter("bp", [128, CPC * HC], F32, isOutput=False)
    bg_d = nc.declare_dram_parameter("bg", [128, CPC * GC], F32, isOutput=False)
    ba_d = nc.declare_dram_parameter("ba", [128, CPC * HC], F32, isOutput=False)
    gsc_d = nc.declare_dram_parameter("gsc", [128, CPC], F32, isOutput=False)
    ident_d = nc.declare_dram_parameter("ident", [128, 128], BF, isOutput=False)
    out_d = nc.declare_dram_parameter("out", [t_steps, H, B], F32, isOutput=True)

    with tile.TileContext(nc) as tc:
        with (
            tc.tile_pool(name="wpool", bufs=1) as wpool,
            tc.tile_pool(name="state", bufs=1) as state,
            tc.tile_pool(name="dram", bufs=1, space="DRAM") as dpool1,
            tc.tile_pool(name="dramr", bufs=2, space="DRAM") as dpool2,
        ):
            # ---- resident SBUF tensors ----------------------------------
            wp_sb = wpool.tile([128, CPC * HC * HC * 128], BF, name="wp_sb")
            wih_sb = wpool.tile([128, CPC * HC * GC * 128], BF, name="wih_sb")
            whh_sb = wpool.tile([128, CPC * HC * GC * 128], BF, name="whh_sb")
            wa_sb = wpool.tile([128, CPC * HC * HC * 128], BF, name="wa_sb")
            w1n_sb = wpool.tile([1, CPC * GC * 128], BF, name="w1n_sb")
            bp_sb = wpool.tile([128, CPC * HC], F32, name="bp_sb")
            bg_sb = wpool.tile([128, CPC * GC], F32, name="bg_sb")
            ba_sb = wpool.tile([128, CPC * HC], F32, name="ba_sb")
            gsc_sb = wpool.tile([128, CPC], F32, name="gsc_sb")
            wproj_sb = wpool.tile([128, EC * HC * 128], BF, name="wproj_sb")
            bproj_sb = wpool.tile([128, HC], F32, name="bproj_sb")
            ident_sb = wpool.tile([128, 128], BF, name="ident_sb")
            ones_col = wpool.tile([128, 1], BF, name="ones_col")
            ones_row = wpool.tile([1, 128], BF, name="ones_row")

            # ping-pong h buffers: gates read old h while h_new is written
            h_st = [
                [state.tile([128, HC, B], BF, name=f"h{c}_{par}") for par in range(2)]
                for c in range(CPC)
            ]
            c_st = [state.tile([128, HC, B], F32, name=f"c{c}") for c in range(CPC)]
            ext = state.tile([128, HC, B], F32, name="ext")
            y_acc = state.tile([128, HC, B], F32, name="y_acc")

            nc.sync.dma_start(wp_sb[:], wp_d[:])
            nc.sync.dma_start(wih_sb[:], wih_d[:])
            nc.sync.dma_start(whh_sb[:], whh_d[:])
            nc.sync.dma_start(wa_sb[:], wa_d[:])
            nc.sync.dma_start(w1n_sb[:], w1n_d[:])
            nc.sync.dma_start(bp_sb[:], bp_d[:])
            nc.sync.dma_start(bg_sb[:], bg_d[:])
            nc.sync.dma_start(ba_sb[:], ba_d[:])
            nc.sync.dma_start(gsc_sb[:], gsc_d[:])
            nc.sync.dma_start(wproj_sb[:], wproj_d[:])
            nc.sync.dma_start(bproj_sb[:], bproj_d[:])
            nc.sync.dma_start(ident_sb[:], ident_d[:])
            nc.vector.memset(ones_col[:], 1.0)
            nc.vector.memset(ones_row[:], 1.0)
            for c in range(CPC):
                nc.vector.memset(h_st[c][0][:], 0.0)
                nc.vector.memset(h_st[c][1][:], 0.0)
                nc.vector.memset(c_st[c][:], 0.0)
            nc.vector.memset(ext[:], 0.0)

            # DRAM staging for xs^T
            xsT_loc = dpool1.tile([TLOC, H, B], BF, name="xsT_loc")
            xsT = dpool1.tile([T, H, B], BF, name="xsT", addr_space="Shared")

            # ---- preamble: embedding gather + projection (sharded by t) --
            with (
                tc.tile_pool(name="prepool", bufs=3) as pre,
                tc.tile_pool(name="preps", bufs=4, space="PSUM") as preps,
                tc.tile_pool(name="preps2", bufs=2, space="PSUM") as preps2,
            ):
                for g in range(NGRP):
                    embT = [
                        pre.tile([128, 512], BF, tag="embT", name=f"embT{g}_{k}")
                        for k in range(EC)
                    ]
                    for tt in range(4):
                        j = g * 4 + tt
                        idx = pre.tile([128, 1], I32, tag="idx", name=f"idx{j}")
                        nc.sync.dma_start(idx[:], tok_d[j])
                        gt = pre.tile([128, E], BF, tag="gt", name=f"gt{j}")
                        nc.gpsimd.indirect_dma_start(
                            out=gt[:],
                            out_offset=None,
                            in_=emb_d[:],
                            in_offset=bass.IndirectOffsetOnAxis(ap=idx[:, 0:1], axis=0),
                        )
                        for k in range(EC):
                            tp = preps.tile([128, 128], BF, tag="tp", name=f"tp{j}_{k}")
                            nc.tensor.transpose(
                                out=tp[:], in_=gt[:, k * 128:(k + 1) * 128],
                                identity=ident_sb[:],
                            )
                            nc.vector.tensor_copy(
                                embT[k][:, tt * 128:(tt + 1) * 128], tp[:]
                            )
                    for m in range(HC):
                        ps_x = preps2.tile([128, 512], F32, tag="psx", name=f"psx{g}_{m}")
                        for k in range(EC):
                            nc.tensor.matmul(
                                ps_x[:],
                                wproj_sb[:, (k * HC + m) * 128:(k * HC + m + 1) * 128],
                                embT[k][:],
                                start=(k == 0),
                                stop=(k == EC - 1),
                            )
                        xsg = pre.tile([128, 512], BF, tag="xsg", name=f"xsg{g}_{m}")
                        nc.scalar.activation(
                            xsg[:], ps_x[:], AF.Identity, bias=bproj_sb[:, m:m + 1]
                        )
                        nc.sync.dma_start(
                            xsT_loc[2 * g, m * 128:(m + 1) * 128, :], xsg[:, 0:B]
                        )
                        nc.sync.dma_start(
                            xsT_loc[2 * g + 1, m * 128:(m + 1) * 128, :], xsg[:, B:2 * B]
                        )

            nc.gpsimd.collective_compute(
                "AllGather",
                ALU.bypass,
                ins=[xsT_loc.opt()],
                outs=[xsT.opt()],
                replica_groups=RG,
            )

            # ---- the scan -----------------------------------------------
            with (
                tc.tile_pool(name="work", bufs=2) as work,
                tc.tile_pool(name="gq", bufs=3) as gqp,
                tc.tile_pool(name="sm", bufs=4) as smp,
                tc.tile_pool(name="ps_pp", bufs=2, space="PSUM") as ps_pp,
                tc.tile_pool(name="ps_gg", bufs=4, space="PSUM") as ps_gg,
                tc.tile_pool(name="ps_ss", bufs=1, space="PSUM") as ps_ss,
                tc.tile_pool(name="ps_pb", bufs=1, space="PSUM") as ps_pb,
            ):
                for t in range(t_steps):
                    xt = work.tile([128, HC, B], BF, tag="xt", name=f"xt{t}")
                    nc.sync.dma_start(
                        xt[:], xsT[t].rearrange("(k p) b -> p k b", p=128)
                    )
                    # x~ = x_t + 0.3 * ext
                    xe = work.tile([128, HC, B], BF, tag="xe", name=f"xe{t}")
                    exb = work.tile([128, HC, B], BF, tag="exb", name=f"exb{t}")
                    for k in range(HC):
                        nc.vector.tensor_scalar_mul(exb[:, k], ext[:, k], 0.3)
                        nc.vector.tensor_add(xe[:, k], exb[:, k], xt[:, k])

                    sbfs, msbfs = [], []
                    ps_list = []
                    for c in range(CPC):
                        # ---- perception matmul + ReLU + LN stats --------
                        p_t = work.tile([128, HC, B], BF, tag="p", name=f"p{t}_{c}")
                        p2 = work.tile([128, B], BF, tag="p2", name=f"p2{t}_{c}")
                        st = ps_ss.tile([64, B], F32, tag="ss", name=f"ss{t}_{c}")
                        for m in range(HC):
                            pp = ps_pp.tile([128, B], F32, tag="pp", name=f"pp{t}_{c}_{m}")
                            for k in range(HC):
                                col = ((c * HC + k) * HC + m) * 128
                                nc.tensor.matmul(
                                    pp[:], wp_sb[:, col:col + 128], xe[:, k],
                                    start=(k == 0), stop=(k == HC - 1),
                                )
                            nc.scalar.activation(
                                p_t[:, m], pp[:], AF.Relu,
                                bias=bp_sb[:, c * HC + m:c * HC + m + 1],
                            )
                            nc.vector.tensor_mul(p2[:], p_t[:, m], p_t[:, m])
                            nc.tensor.matmul(
                                st[0:1, :], ones_col[:], p_t[:, m],
                                start=(m == 0), stop=(m == HC - 1),
                            )
                            nc.tensor.matmul(
                                st[32:33, :], ones_col[:], p2[:],
                                start=(m == 0), stop=(m == HC - 1),
                            )
                        # ---- LN smalls ----------------------------------
                        mu = smp.tile([1, B], F32, tag="mu", name=f"mu{t}_{c}")
                        vpe = smp.tile([1, B], F32, tag="vpe", name=f"vpe{t}_{c}")
                        v_ = smp.tile([1, B], F32, tag="v", name=f"v{t}_{c}")
                        sig = smp.tile([1, B], F32, tag="sig", name=f"sig{t}_{c}")
                        s_ = smp.tile([1, B], F32, tag="s", name=f"s{t}_{c}")
                        s_bf = smp.tile([1, B], BF, tag="sbf", name=f"sbf{t}_{c}")
                        ms_bf = smp.tile([1, B], BF, tag="msbf", name=f"msbf{t}_{c}")
                        nc.vector.tensor_scalar_mul(mu[:], st[0:1, :], 1.0 / H)
                        nc.vector.tensor_scalar(
                            vpe[:], st[32:33, :], 1.0 / H, LN_EPS, ALU.mult, ALU.add
                        )
                        musq = smp.tile([1, B], F32, tag="musq", name=f"musq{t}_{c}")
                        nc.vector.tensor_mul(musq[:], mu[:], mu[:])
                        nc.vector.tensor_sub(v_[:], vpe[:], musq[:])
                        nc.scalar.activation(sig[:], v_[:], AF.Sqrt)
                        nc.vector.reciprocal_approx_fast(out=s_[:], in_=sig[:])
                        nc.vector.tensor_copy(s_bf[:], s_[:])
                        nc.vector.tensor_mul(ms_bf[:], mu[:], s_[:])
                        # broadcast rstd across partitions via rank-1 matmul
                        pb = ps_pb.tile([128, B], F32, tag="pb", name=f"pb{t}_{c}")
                        nc.tensor.matmul(
                            pb[:], ones_row[:], s_bf[:], start=True, stop=True
                        )
                        sb_bf = work.tile([128, B], BF, tag="sbb", name=f"sbb{t}_{c}")
                        nc.vector.tensor_copy(sb_bf[:], pb[:])
                        p_s = work.tile([128, HC, B], BF, tag="psld", name=f"psld{t}_{c}")
                        for m in range(HC):
                            nc.vector.tensor_mul(p_s[:, m], p_t[:, m], sb_bf[:])
                        sbfs.append(s_bf)
                        msbfs.append(ms_bf)
                        ps_list.append(p_s)

                        if dbg and t == 0 and c == 0:
                            dx = work.tile([128, HC, B], F32, tag="dx", name="dx")
                            for kk in range(HC):
                                nc.vector.tensor_copy(dx[:, kk], xe[:, kk])
                            nc.sync.dma_start(
                                dbg_d[0].rearrange("(k p) b -> p k b", p=128), dx[:])
                            for kk in range(HC):
                                nc.vector.tensor_copy(dx[:, kk], p_t[:, kk])
                            nc.sync.dma_start(
                                dbg_d[1].rearrange("(k p) b -> p k b", p=128), dx[:])
                            for kk in range(HC):
                                nc.vector.tensor_copy(dx[:, kk], p_s[:, kk])
                            nc.sync.dma_start(
                                dbg_d[2].rearrange("(k p) b -> p k b", p=128), dx[:])
                            # stats rows: mu, vpe, v, sig, s into dbg[3, 0:5, :]
                            for ii, tt_ in enumerate([mu, vpe, v_, sig, s_]):
                                nc.sync.dma_start(dbg_d[3, ii:ii + 1, :], tt_[:])
                            nc.vector.tensor_copy(dx[:, 0], sb_bf[:])
                            nc.sync.dma_start(dbg_d[4, 0:128, :], dx[:, 0])

                    for c in range(CPC):
                        p_s = ps_list[c]
                        ms_bf = msbfs[c]
                        hr = h_st[c][t % 2]
                        hw = h_st[c][(t + 1) % 2]
                        # ---- gates + LSTM cell --------------------------
                        for j in range(HC):
                            gq = [None] * 4
                            for gi in range(4):
                                mg = gi * HC + j
                                gg = ps_gg.tile(
                                    [128, B], F32, tag="gg", name=f"gg{t}_{c}_{mg}"
                                )
                                for k in range(HC):
                                    col = ((c * HC + k) * GC + mg) * 128
                                    nc.tensor.matmul(
                                        gg[:], wih_sb[:, col:col + 128], p_s[:, k],
                                        start=(k == 0), stop=False,
                                    )
                                for k in range(HC):
                                    col = ((c * HC + k) * GC + mg) * 128
                                    nc.tensor.matmul(
                                        gg[:], whh_sb[:, col:col + 128], hr[:, k],
                                        start=False, stop=False,
                                    )
                                col1 = (c * GC + mg) * 128
                                nc.tensor.matmul(
                                    gg[:], w1n_sb[0:1, col1:col1 + 128], ms_bf[:],
                                    start=False, stop=True,
                                )
                                func = AF.Tanh if gi == 2 else AF.Sigmoid
                                dt_g = BF if gi == 3 else F32
                                gq[gi] = gqp.tile(
                                    [128, B], dt_g, tag=f"gq{gi}", name=f"gq{t}_{c}_{mg}"
                                )
                                nc.scalar.activation(
                                    gq[gi][:], gg[:], func,
                                    bias=bg_sb[:, c * GC + mg:c * GC + mg + 1],
                                )
                            t1 = gqp.tile([128, B], F32, tag="t1", name=f"t1{t}_{c}_{j}")
                            t2 = gqp.tile([128, B], F32, tag="t2", name=f"t2{t}_{c}_{j}")
                            nc.vector.tensor_mul(t1[:], gq[0][:], gq[2][:])
                            nc.vector.tensor_mul(t2[:], gq[1][:], c_st[c][:, j])
                            nc.vector.tensor_add(c_st[c][:, j], t1[:], t2[:])
                            tc_ = gqp.tile([128, B], BF, tag="tc", name=f"tc{t}_{c}_{j}")
                            nc.scalar.activation(tc_[:], c_st[c][:, j], AF.Tanh)
                            nc.vector.tensor_mul(hw[:, j], gq[3][:], tc_[:])
                        # ---- association + gated accumulate -------------
                        for m in range(HC):
                            pa = ps_pp.tile([128, B], F32, tag="pp", name=f"pa{t}_{c}_{m}")
                            for k in range(HC):
                                col = ((c * HC + k) * HC + m) * 128
                                nc.tensor.matmul(
                                    pa[:], wa_sb[:, col:col + 128], hw[:, k],
                                    start=(k == 0), stop=(k == HC - 1),
                                )
                            a_ = gqp.tile([128, B], F32, tag="a", name=f"a{t}_{c}_{m}")
                            nc.scalar.activation(
                                a_[:], pa[:], AF.Tanh,
                                bias=ba_sb[:, c * HC + m:c * HC + m + 1],
                            )
                            if c == 0:
                                nc.vector.tensor_scalar_mul(
                                    y_acc[:, m], a_[:], gsc_sb[:, 0:1]
                                )
                            else:
                                nc.vector.scalar_tensor_tensor(
                                    y_acc[:, m], a_[:], gsc_sb[:, c:c + 1],
                                    y_acc[:, m], ALU.mult, ALU.add,
                                )

                    if dbg and t == 0:
                        dx2 = work.tile([128, HC, B], F32, tag="dx", name="dx2")
                        for kk in range(HC):
                            nc.vector.tensor_copy(dx2[:, kk], c_st[0][:, kk])
                        nc.sync.dma_start(
                            dbg_d[5].rearrange("(k p) b -> p k b", p=128), dx2[:])
                        for kk in range(HC):
                            nc.vector.tensor_copy(dx2[:, kk], h_st[0][(t + 1) % 2][:, kk])
                        nc.sync.dma_start(
                            dbg_d[6].rearrange("(k p) b -> p k b", p=128), dx2[:])
                        for kk in range(HC):
                            nc.vector.tensor_copy(dx2[:, kk], y_acc[:, kk])
                        nc.sync.dma_start(
                            dbg_d[7].rearrange("(k p) b -> p k b", p=128), dx2[:])

                    # ---- cross-cell mean via AllReduce ------------------
                    ar_i = dpool2.tile([H, B], F32, tag="ari", name=f"ari{t}")
                    ar_o = dpool2.tile(
                        [H, B], F32, tag="aro", name=f"aro{t}", addr_space="Shared"
                    )
                    nc.sync.dma_start(
                        ar_i.rearrange("(k p) b -> p k b", p=128), y_acc[:]
                    )
                    nc.gpsimd.collective_compute(
                        "AllReduce",
                        ALU.add,
                        ins=[ar_i.opt()],
                        outs=[ar_o.opt()],
                        replica_groups=RG,
                    )
                    if t < t_steps - 1:
                        nc.sync.dma_start(
                            ext[:], ar_o.rearrange("(k p) b -> p k b", p=128)
                        )
                    nc.sync.dma_start(out_d[t], ar_o[:])

    nc.compile()
    return nc


def prepare_inputs(tokens, emb, Wproj, bproj, Wp, bp, ln_g, ln_b,
                   Wih, bih, Whh, bhh, Wa, ba, gate_logit):
    """Host-side parameter prep + per-core sharding. Returns in_maps."""
    tokens = np.asarray(tokens).astype(np.int32)
    emb = np.asarray(emb, dtype=np.float32).copy()
    emb[0] = 0.0  # padding_idx
    emb_bf = emb.astype(BF16)

    Wproj = np.asarray(Wproj, np.float32)
    bproj = np.asarray(bproj, np.float32)
    Wp = np.asarray(Wp, np.float32)
    bp = np.asarray(bp, np.float32)
    ln_g = np.asarray(ln_g, np.float32)
    ln_b = np.asarray(ln_b, np.float32)
    Wih = np.asarray(Wih, np.float32)
    bih = np.asarray(bih, np.float32)
    Whh = np.asarray(Whh, np.float32)
    bhh = np.asarray(bhh, np.float32)
    Wa = np.asarray(Wa, np.float32)
    ba = np.asarray(ba, np.float32)
    gate_logit = np.asarray(gate_logit, np.float32)

    # Fold the LN affine (g, b) into the input-hidden weights / gate bias.
    Wih_g = Wih * ln_g[:, None, :]                       # [C, 4H, H]
    bg = bih + np.einsum("cgh,ch->cg", Wih, ln_b) + bhh  # [C, 4H]
    w1n = -Wih_g.sum(-1)                                 # [C, 4H]
    gsc = 1.0 / (1.0 + np.exp(-gate_logit)) / C          # [C]

    wproj_p = _pack_lhsT(Wproj).astype(BF16)
    bproj_p = _pack_bias(bproj[None, :])                 # [128, 4]
    ident = np.eye(128, dtype=np.float32).astype(BF16)

    in_maps = []
    for i in range(NCORES):
        cs = slice(CPC * i, CPC * (i + 1))
        wp_p = np.concatenate([_pack_lhsT(Wp[c]) for c in range(cs.start, cs.stop)], 1)
        wih_p = np.concatenate(
            [_pack_lhsT(np.ascontiguousarray(Wih_g[c].T)) for c in range(cs.start, cs.stop)], 1
        )
        whh_p = np.concatenate(
            [_pack_lhsT(np.ascontiguousarray(Whh[c].T)) for c in range(cs.start, cs.stop)], 1
        )
        wa_p = np.concatenate([_pack_lhsT(Wa[c]) for c in range(cs.start, cs.stop)], 1)

        t0 = TLOC * i
        tok_core = np.ascontiguousarray(
            tokens[:, t0:t0 + TLOC].T.reshape(NGRP * 4, 128, 1)
        )

        in_maps.append({
            "emb": emb_bf,
            "tok": tok_core,
            "wproj": wproj_p,
            "bproj": bproj_p,
            "wp": wp_p.astype(BF16),
            "wih": wih_p.astype(BF16),
            "whh": whh_p.astype(BF16),
            "wa": wa_p.astype(BF16),
            "w1n": w1n[cs].reshape(1, -1).astype(BF16),
            "bp": _pack_bias(bp[cs]),
            "bg": _pack_bias(bg[cs]),
            "ba": _pack_bias(ba[cs]),
            "gsc": np.broadcast_to(gsc[cs], (128, CPC)).astype(np.float32).copy(),
            "ident": ident,
        })
    return in_maps


_CACHE = {}


def run(inputs: dict, t_steps: int = T, trace: bool = False):
    key = t_steps
    if key not in _CACHE:
        _CACHE[key] = build_program(t_steps)
    nc = _CACHE[key]
    in_maps = prepare_inputs(**inputs)
    res = run_bass_kernel_spmd(nc, in_maps, list(range(NCORES)), trace=trace)
    ysT = res.results[0]["out"]  # [t_steps, H, B] f32
    out = np.ascontiguousarray(ysT.transpose(2, 0, 1))  # [B, t_steps, H]
    return out, res


def kernel(**inputs) -> np.ndarray:
    out, _ = run(inputs, T)
    return out


def run_timed(inputs: dict, t_steps: int = T, n_iters: int = 3):
    """Replicates bass2jax.run_bass_via_pjrt's multi-core path but keeps the
    jitted executable and device-resident inputs so repeat calls measure the
    on-device execution time (plus dispatch) rather than NEFF compile or
    host->device transfer."""
    import time
    import jax
    from jax.sharding import Mesh, PartitionSpec
    from jax.experimental.shard_map import shard_map
    from concourse import bass2jax, mybir as _mb

    key = t_steps
    if key not in _CACHE:
        _CACHE[key] = build_program(t_steps)
    nc = _CACHE[key]
    in_maps = prepare_inputs(**inputs)

    bass2jax.install_neuronx_cc_hook()
    part_name = nc.partition_id_tensor.name if nc.partition_id_tensor else None
    in_names, out_names, out_avals, zero_outs = [], [], [], []
    for alloc in nc.m.functions[0].allocations:
        if not isinstance(alloc, _mb.MemoryLocationSet):
            continue
        name = alloc.memorylocations[0].name
        if alloc.kind == "ExternalInput":
            if name != part_name:
                in_names.append(name)
        elif alloc.kind == "ExternalOutput":
            out_names.append(name)
            out_avals.append(
                jax.core.ShapedArray(alloc.tensor_shape, _mb.dt.np(alloc.dtype))
            )
            zero_outs.append(
                np.zeros(alloc.tensor_shape, dtype=_mb.dt.np(alloc.dtype))
            )
    n_params = len(in_names)
    all_names = in_names + out_names
    if part_name is not None:
        all_names.append(part_name)

    def _body(*args):
        operands = list(args)
        if part_name is not None:
            operands.append(bass2jax.partition_id_tensor())
        outs = bass2jax._bass_exec_p.bind(
            *operands,
            out_avals=tuple(out_avals),
            in_names=tuple(all_names),
            out_names=tuple(out_names),
            lowering_input_output_aliases=(),
            sim_require_finite=True,
            sim_require_nnan=True,
            nc=nc,
        )
        return tuple(outs)

    devices = jax.devices()[:NCORES]
    mesh = Mesh(np.asarray(devices), ("core",))
    n_outs = len(out_names)
    sharded = jax.jit(
        shard_map(
            _body, mesh=mesh,
            in_specs=(PartitionSpec("core"),) * (n_params + n_outs),
            out_specs=(PartitionSpec("core"),) * n_outs,
            check_rep=False,
        ),
        keep_unused=True,
    )
    concat_in = [
        np.concatenate([np.asarray(in_maps[c][nm]) for c in range(NCORES)], axis=0)
        for nm in in_names
    ]
    concat_zeros = [
        np.zeros((NCORES * z.shape[0], *z.shape[1:]), z.dtype) for z in zero_outs
    ]
    sh = jax.sharding.NamedSharding(mesh, PartitionSpec("core"))
    dev_in = [jax.device_put(a, sh) for a in concat_in]
    dev_zero = [jax.device_put(a, sh) for a in concat_zeros]
    out_arrs = sharded(*dev_in, *dev_zero)  # warm-up / compile
    jax.block_until_ready(out_arrs)
    # pipeline n_iters calls without intermediate blocking to amortize the
    # axon dispatch round-trip; calls serialize on the devices.
    n_pipe = max(n_iters, 12)
    t0 = time.perf_counter()
    rs = [sharded(*dev_in, *dev_zero) for _ in range(n_pipe)]
    jax.block_until_ready(rs)
    per_call = (time.perf_counter() - t0) / n_pipe
    idx = out_names.index("out")
    ysT = np.asarray(out_arrs[idx]).reshape(NCORES, *out_avals[idx].shape)[0]
    out = np.ascontiguousarray(ysT.transpose(2, 0, 1))
    return out, per_call


# revision 17
# speedup vs baseline: 1.1448x; 1.1448x over previous
"""Trainium2 Bass kernel for nn_CognitiveNetwork (16-cell LSTM message-passing net).

Strategy
--------
* Expert-parallel over the C=16 cells: 2 cells per NeuronCore.  All weights
  then fit in SBUF (bf16) and stay resident for the whole scan -- no per-step
  weight traffic (the problem's memory bottleneck).
* Fully "transposed" dataflow: activations live as [H, B] (H on partitions),
  so every bias is a per-partition vector (free via the ACT engine's bias
  operand) and no on-device activation transposes are needed.
* LayerNorm: Sum(p) / Sum(p^2) via ones-vector matmuls on the PE;
  rstd = 1/sqrt(var+eps) via ACT Sqrt + DVE reciprocal_approx_fast;
  the (p - mu) term is folded into the gates matmul as a rank-1 correction
  (lhsT = -rowsum(Wih), rhs = mu*rstd), so normalization costs one DVE
  multiply (p * rstd_broadcast) plus 17 tiny matmuls.
* Per-step cell-mean y: local partial sum, 512KB fp32 AllReduce across the 8
  cores; y feeds the next step's external injection (x_t + 0.3*y).
* Embedding gather + input projection: sharded over cores by timestep (16 t's
  per core), indirect-DMA row gather from the bf16 embedding table, PE
  transpose, projection matmul, then one AllGather of xs^T [T, H, B] bf16.
"""

import os
import sys

sys.path.insert(0, "/opt/trn_rl_repo")

import numpy as np
import ml_dtypes

from concourse import bass, bacc, mybir, tile
from concourse.bass_utils import run_bass_kernel_spmd

BF16 = ml_dtypes.bfloat16

# Problem constants (hardcoded per contract).
V, E, H, C = 50257, 256, 512, 16
B, T = 256, 128
LN_EPS = 1e-5

NCORES = 8
CPC = C // NCORES        # cells per core = 2
HC = H // 128            # h chunks = 4
EC = E // 128            # e chunks = 2
GC = (4 * H) // 128      # gate chunks = 16
TLOC = T // NCORES       # timesteps gathered per core = 16
NGRP = TLOC // 2         # preamble groups per core (2 t's = 512 tokens each)

F32 = mybir.dt.float32
BF = mybir.dt.bfloat16
I32 = mybir.dt.int32
AF = mybir.ActivationFunctionType
ALU = mybir.AluOpType
RG = [list(range(NCORES))]


def _pack_lhsT(w: np.ndarray) -> np.ndarray:
    """Pack [K, M] weight into SBUF lhsT layout [128, (K/128)*(M/128)*128].

    Column block index (k*mc + m)*128 + j holds w[k*128 + p, m*128 + j] at
    partition p.
    """
    K, M = w.shape
    kc, mc = K // 128, M // 128
    return np.ascontiguousarray(
        w.reshape(kc, 128, mc, 128).transpose(1, 0, 2, 3).reshape(128, kc * mc * 128)
    )


def _pack_bias(b: np.ndarray) -> np.ndarray:
    """[n, M] -> [128, n*(M/128)]: column n*idx... (cell-major, chunk-minor)."""
    n, M = b.shape
    mc = M // 128
    return np.ascontiguousarray(
        b.reshape(n, mc, 128).transpose(2, 0, 1).reshape(128, n * mc)
    )


def build_program(t_steps: int = T, dbg: bool = False):
    nc = bacc.Bacc(
        "TRN2",
        target_bir_lowering=False,
        debug=False,
        num_devices=NCORES,
    )
    dbg_d = (
        nc.declare_dram_parameter("dbg", [8, H, B], F32, isOutput=True)
        if dbg else None
    )

    # ---- I/O -------------------------------------------------------------
    emb_d = nc.declare_dram_parameter("emb", [V, E], BF, isOutput=False)
    tok_d = nc.declare_dram_parameter("tok", [NGRP * 4, 128, 1], I32, isOutput=False)
    wproj_d = nc.declare_dram_parameter("wproj", [128, EC * HC * 128], BF, isOutput=False)
    bproj_d = nc.declare_dram_parameter("bproj", [128, HC], F32, isOutput=False)
    wp_d = nc.declare_dram_parameter("wp", [128, CPC * HC * HC * 128], BF, isOutput=False)
    wih_d = nc.declare_dram_parameter("wih", [128, CPC * HC * GC * 128], BF, isOutput=False)
    whh_d = nc.declare_dram_parameter("whh", [128, CPC * HC * GC * 128], BF, isOutput=False)
    wa_d = nc.declare_dram_parameter("wa", [128, CPC * HC * HC * 128], BF, isOutput=False)
    w1n_d = nc.declare_dram_parameter("w1n", [1, CPC * GC * 128], BF, isOutput=False)
    bp_d = nc.declare_dram_parameter("bp", [128, CPC * HC], F32, isOutput=False)
    bg_d = nc.declare_dram_parameter("bg", [128, CPC * GC], F32, isOutput=False)
    ba_d = nc.declare_dram_parameter("ba", [128, CPC * HC], F32, isOutput=False)
    gsc_d = nc.declare_dram_parameter("gsc", [128, CPC], F32, isOutput=False)
    ident_d = nc.declare_dram_parameter("ident", [128, 128], BF, isOutput=False)
    out_d = nc.declare_dram_parameter("out", [t_steps, H, B], F32, isOutput=True)

    with tile.TileContext(nc) as tc:
        with (
            tc.tile_pool(name="wpool", bufs=1) as wpool,
            tc.tile_pool(name="state", bufs=1) as state,
            tc.tile_pool(name="dram", bufs=1, space="DRAM") as dpool1,
            tc.tile_pool(name="dramr", bufs=2, space="DRAM") as dpool2,
        ):
            # ---- resident SBUF tensors ----------------------------------
            wp_sb = wpool.tile([128, CPC * HC * HC * 128], BF, name="wp_sb")
            wih_sb = wpool.tile([128, CPC * HC * GC * 128], BF, name="wih_sb")
            whh_sb = wpool.tile([128, CPC * HC * GC * 128], BF, name="whh_sb")
            wa_sb = wpool.tile([128, CPC * HC * HC * 128], BF, name="wa_sb")
            w1n_sb = wpool.tile([1, CPC * GC * 128], BF, name="w1n_sb")
            bp_sb = wpool.tile([128, CPC * HC], F32, name="bp_sb")
            bg_sb = wpool.tile([128, CPC * GC], F32, name="bg_sb")
            ba_sb = wpool.tile([128, CPC * HC], F32, name="ba_sb")
            gsc_sb = wpool.tile([128, CPC], F32, name="gsc_sb")
            wproj_sb = wpool.tile([128, EC * HC * 128], BF, name="wproj_sb")
            bproj_sb = wpool.tile([128, HC], F32, name="bproj_sb")
            ident_sb = wpool.tile([128, 128], BF, name="ident_sb")
            ones_col = wpool.tile([128, 1], BF, name="ones_col")
            ones_row = wpool.tile([1, 128], BF, name="ones_row")

            # ping-pong h buffers: gates read old h while h_new is written
            h_st = [
                [state.tile([128, HC, B], BF, name=f"h{c}_{par}") for par in range(2)]
                for c in range(CPC)
            ]
            c_st = [state.tile([128, HC, B], F32, name=f"c{c}") for c in range(CPC)]
            ext = state.tile([128, HC, B], F32, name="ext")
            y_acc = state.tile([128, HC, B], F32, name="y_acc")

            nc.sync.dma_start(wp_sb[:], wp_d[:])
            nc.sync.dma_start(wih_sb[:], wih_d[:])
            nc.sync.dma_start(whh_sb[:], whh_d[:])
            nc.sync.dma_start(wa_sb[:], wa_d[:])
            nc.sync.dma_start(w1n_sb[:], w1n_d[:])
            nc.sync.dma_start(bp_sb[:], bp_d[:])
            nc.sync.dma_start(bg_sb[:], bg_d[:])
            nc.sync.dma_start(ba_sb[:], ba_d[:])
            nc.sync.dma_start(gsc_sb[:], gsc_d[:])
            nc.sync.dma_start(wproj_sb[:], wproj_d[:])
            nc.sync.dma_start(bproj_sb[:], bproj_d[:])
            nc.sync.dma_start(ident_sb[:], ident_d[:])
            nc.vector.memset(ones_col[:], 1.0)
            nc.vector.memset(ones_row[:], 1.0)
            for c in range(CPC):
                nc.vector.memset(h_st[c][0][:], 0.0)
                nc.vector.memset(h_st[c][1][:], 0.0)
                nc.vector.memset(c_st[c][:], 0.0)
            nc.vector.memset(ext[:], 0.0)

            # DRAM staging for xs^T
            xsT_loc = dpool1.tile([TLOC, H, B], BF, name="xsT_loc")
            xsT = dpool1.tile([T, H, B], BF, name="xsT", addr_space="Shared")

            # ---- preamble: embedding gather + projection (sharded by t) --
            with (
                tc.tile_pool(name="prepool", bufs=3) as pre,
                tc.tile_pool(name="preps", bufs=4, space="PSUM") as preps,
                tc.tile_pool(name="preps2", bufs=2, space="PSUM") as preps2,
            ):
                for g in range(NGRP):
                    embT = [
                        pre.tile([128, 512], BF, tag="embT", name=f"embT{g}_{k}")
                        for k in range(EC)
                    ]
                    for tt in range(4):
                        j = g * 4 + tt
                        idx = pre.tile([128, 1], I32, tag="idx", name=f"idx{j}")
                        nc.sync.dma_start(idx[:], tok_d[j])
                        gt = pre.tile([128, E], BF, tag="gt", name=f"gt{j}")
                        nc.gpsimd.indirect_dma_start(
                            out=gt[:],
                            out_offset=None,
                            in_=emb_d[:],
                            in_offset=bass.IndirectOffsetOnAxis(ap=idx[:, 0:1], axis=0),
                        )
                        for k in range(EC):
                            tp = preps.tile([128, 128], BF, tag="tp", name=f"tp{j}_{k}")
                            nc.tensor.transpose(
                                out=tp[:], in_=gt[:, k * 128:(k + 1) * 128],
                                identity=ident_sb[:],
                            )
                            nc.vector.tensor_copy(
                                embT[k][:, tt * 128:(tt + 1) * 128], tp[:]
                            )
                    for m in range(HC):
                        ps_x = preps2.tile([128, 512], F32, tag="psx", name=f"psx{g}_{m}")
                        for k in range(EC):
                            nc.tensor.matmul(
                                ps_x[:],
                                wproj_sb[:, (k * HC + m) * 128:(k * HC + m + 1) * 128],
                                embT[k][:],
                                start=(k == 0),
                                stop=(k == EC - 1),
                            )
                        xsg = pre.tile([128, 512], BF, tag="xsg", name=f"xsg{g}_{m}")
                        nc.scalar.activation(
                            xsg[:], ps_x[:], AF.Identity, bias=bproj_sb[:, m:m + 1]
                        )
                        nc.sync.dma_start(
                            xsT_loc[2 * g, m * 128:(m + 1) * 128, :], xsg[:, 0:B]
                        )
                        nc.sync.dma_start(
                            xsT_loc[2 * g + 1, m * 128:(m + 1) * 128, :], xsg[:, B:2 * B]
                        )

            nc.gpsimd.collective_compute(
                "AllGather",
                ALU.bypass,
                ins=[xsT_loc.opt()],
                outs=[xsT.opt()],
                replica_groups=RG,
            )

            # ---- the scan -----------------------------------------------
            with (
                tc.tile_pool(name="work", bufs=2) as work,
                tc.tile_pool(name="gq", bufs=2) as gqp,
                tc.tile_pool(name="sm", bufs=2) as smp,
                tc.tile_pool(name="ps_pp", bufs=2, space="PSUM") as ps_pp,
                tc.tile_pool(name="ps_gg", bufs=4, space="PSUM") as ps_gg,
                tc.tile_pool(name="ps_ss", bufs=1, space="PSUM") as ps_ss,
                tc.tile_pool(name="ps_pb", bufs=1, space="PSUM") as ps_pb,
            ):
                for t in range(t_steps):
                    xt = work.tile([128, HC, B], BF, tag="xt", name=f"xt{t}")
                    nc.sync.dma_start(
                        xt[:], xsT[t].rearrange("(k p) b -> p k b", p=128)
                    )
                    # x~ = x_t + 0.3 * ext
                    xe = work.tile([128, HC, B], BF, tag="xe", name=f"xe{t}")
                    exb = work.tile([128, HC, B], BF, tag="exb", name=f"exb{t}")
                    nc.vector.tensor_scalar_mul(exb[:], ext[:], 0.3)
                    nc.vector.tensor_add(xe[:], exb[:], xt[:])

                    sbfs, msbfs = [], []
                    ps_list = []
                    for c in range(CPC):
                        # ---- perception matmul + ReLU + LN stats --------
                        p_t = work.tile([128, HC, B], BF, tag="p", name=f"p{t}_{c}")
                        p2 = work.tile([128, HC, B], BF, tag="p2", name=f"p2{t}_{c}")
                        st = ps_ss.tile([64, B], F32, tag="ss", name=f"ss{t}_{c}")
                        for m in range(HC):
                            pp = ps_pp.tile([128, B], F32, tag="pp", name=f"pp{t}_{c}_{m}")
                            for k in range(HC):
                                col = ((c * HC + k) * HC + m) * 128
                                nc.tensor.matmul(
                                    pp[:], wp_sb[:, col:col + 128], xe[:, k],
                                    start=(k == 0), stop=(k == HC - 1),
                                )
                            nc.scalar.activation(
                                p_t[:, m], pp[:], AF.Relu,
                                bias=bp_sb[:, c * HC + m:c * HC + m + 1],
                            )
                        nc.vector.tensor_mul(p2[:], p_t[:], p_t[:])
                        for m in range(HC):
                            nc.tensor.matmul(
                                st[0:1, :], ones_col[:], p_t[:, m],
                                start=(m == 0), stop=(m == HC - 1),
                            )
                            nc.tensor.matmul(
                                st[32:33, :], ones_col[:], p2[:, m],
                                start=(m == 0), stop=(m == HC - 1),
                            )
                        # ---- LN smalls ----------------------------------
                        mu = smp.tile([1, B], F32, tag="mu", name=f"mu{t}_{c}")
                        vpe = smp.tile([1, B], F32, tag="vpe", name=f"vpe{t}_{c}")
                        v_ = smp.tile([1, B], F32, tag="v", name=f"v{t}_{c}")
                        sig = smp.tile([1, B], F32, tag="sig", name=f"sig{t}_{c}")
                        s_ = smp.tile([1, B], F32, tag="s", name=f"s{t}_{c}")
                        s_bf = smp.tile([1, B], BF, tag="sbf", name=f"sbf{t}_{c}")
                        ms_bf = smp.tile([1, B], BF, tag="msbf", name=f"msbf{t}_{c}")
                        nc.vector.tensor_scalar_mul(mu[:], st[0:1, :], 1.0 / H)
                        nc.vector.tensor_scalar(
                            vpe[:], st[32:33, :], 1.0 / H, LN_EPS, ALU.mult, ALU.add
                        )
                        musq = smp.tile([1, B], F32, tag="musq", name=f"musq{t}_{c}")
                        nc.vector.tensor_mul(musq[:], mu[:], mu[:])
                        nc.vector.tensor_sub(v_[:], vpe[:], musq[:])
                        nc.scalar.activation(sig[:], v_[:], AF.Sqrt)
                        nc.vector.reciprocal_approx_fast(out=s_[:], in_=sig[:])
                        nc.vector.tensor_copy(s_bf[:], s_[:])
                        nc.vector.tensor_mul(ms_bf[:], mu[:], s_[:])
                        # broadcast rstd across partitions via rank-1 matmul
                        pb = ps_pb.tile([128, B], F32, tag="pb", name=f"pb{t}_{c}")
                        nc.tensor.matmul(
                            pb[:], ones_row[:], s_bf[:], start=True, stop=True
                        )
                        sb_bf = work.tile([128, B], BF, tag="sbb", name=f"sbb{t}_{c}")
                        nc.vector.tensor_copy(sb_bf[:], pb[:])
                        p_s = work.tile([128, HC, B], BF, tag="psld", name=f"psld{t}_{c}")
                        for m in range(HC):
                            nc.vector.tensor_mul(p_s[:, m], p_t[:, m], sb_bf[:])
                        sbfs.append(s_bf)
                        msbfs.append(ms_bf)
                        ps_list.append(p_s)

                        if dbg and t == 0 and c == 0:
                            dx = work.tile([128, HC, B], F32, tag="dx", name="dx")
                            for kk in range(HC):
                                nc.vector.tensor_copy(dx[:, kk], xe[:, kk])
                            nc.sync.dma_start(
                                dbg_d[0].rearrange("(k p) b -> p k b", p=128), dx[:])
                            for kk in range(HC):
                                nc.vector.tensor_copy(dx[:, kk], p_t[:, kk])
                            nc.sync.dma_start(
                                dbg_d[1].rearrange("(k p) b -> p k b", p=128), dx[:])
                            for kk in range(HC):
                                nc.vector.tensor_copy(dx[:, kk], p_s[:, kk])
                            nc.sync.dma_start(
                                dbg_d[2].rearrange("(k p) b -> p k b", p=128), dx[:])
                            # stats rows: mu, vpe, v, sig, s into dbg[3, 0:5, :]
                            for ii, tt_ in enumerate([mu, vpe, v_, sig, s_]):
                                nc.sync.dma_start(dbg_d[3, ii:ii + 1, :], tt_[:])
                            nc.vector.tensor_copy(dx[:, 0], sb_bf[:])
                            nc.sync.dma_start(dbg_d[4, 0:128, :], dx[:, 0])

                    for c in range(CPC):
                        p_s = ps_list[c]
                        ms_bf = msbfs[c]
                        hr = h_st[c][t % 2]
                        hw = h_st[c][(t + 1) % 2]
                        # ---- gates + LSTM cell --------------------------
                        gq = [
                            gqp.tile([128, HC, B], BF if gi == 3 else F32,
                                     tag=f"gq{gi}", name=f"gq{t}_{c}_{gi}")
                            for gi in range(4)
                        ]
                        for j in range(HC):
                            for gi in range(4):
                                mg = gi * HC + j
                                gg = ps_gg.tile(
                                    [128, B], F32, tag="gg", name=f"gg{t}_{c}_{mg}"
                                )
                                for k in range(HC):
                                    col = ((c * HC + k) * GC + mg) * 128
                                    nc.tensor.matmul(
                                        gg[:], wih_sb[:, col:col + 128], p_s[:, k],
                                        start=(k == 0), stop=False,
                                    )
                                for k in range(HC):
                                    col = ((c * HC + k) * GC + mg) * 128
                                    nc.tensor.matmul(
                                        gg[:], whh_sb[:, col:col + 128], hr[:, k],
                                        start=False, stop=False,
                                    )
                                col1 = (c * GC + mg) * 128
                                nc.tensor.matmul(
                                    gg[:], w1n_sb[0:1, col1:col1 + 128], ms_bf[:],
                                    start=False, stop=True,
                                )
                                func = AF.Tanh if gi == 2 else AF.Sigmoid
                                nc.scalar.activation(
                                    gq[gi][:, j], gg[:], func,
                                    bias=bg_sb[:, c * GC + mg:c * GC + mg + 1],
                                )
                        t1 = gqp.tile([128, HC, B], F32, tag="t1", name=f"t1{t}_{c}")
                        t2 = gqp.tile([128, HC, B], F32, tag="t2", name=f"t2{t}_{c}")
                        nc.vector.tensor_mul(t1[:], gq[0][:], gq[2][:])
                        nc.vector.tensor_mul(t2[:], gq[1][:], c_st[c][:])
                        nc.vector.tensor_add(c_st[c][:], t1[:], t2[:])
                        tc_ = gqp.tile([128, HC, B], BF, tag="tc", name=f"tc{t}_{c}")
                        nc.scalar.activation(tc_[:], c_st[c][:], AF.Tanh)
                        nc.vector.tensor_mul(hw[:], gq[3][:], tc_[:])
                        # ---- association + gated accumulate -------------
                        for m in range(HC):
                            pa = ps_pp.tile([128, B], F32, tag="pp", name=f"pa{t}_{c}_{m}")
                            for k in range(HC):
                                col = ((c * HC + k) * HC + m) * 128
                                nc.tensor.matmul(
                                    pa[:], wa_sb[:, col:col + 128], hw[:, k],
                                    start=(k == 0), stop=(k == HC - 1),
                                )
                            a_ = gqp.tile([128, B], F32, tag="a", name=f"a{t}_{c}_{m}")
                            nc.scalar.activation(
                                a_[:], pa[:], AF.Tanh,
                                bias=ba_sb[:, c * HC + m:c * HC + m + 1],
                            )
                            if c == 0:
                                nc.vector.tensor_scalar_mul(
                                    y_acc[:, m], a_[:], gsc_sb[:, 0:1]
                                )
                            else:
                                nc.vector.scalar_tensor_tensor(
                                    y_acc[:, m], a_[:], gsc_sb[:, c:c + 1],
                                    y_acc[:, m], ALU.mult, ALU.add,
                                )

                    if dbg and t == 0:
                        dx2 = work.tile([128, HC, B], F32, tag="dx", name="dx2")
                        for kk in range(HC):
                            nc.vector.tensor_copy(dx2[:, kk], c_st[0][:, kk])
                        nc.sync.dma_start(
                            dbg_d[5].rearrange("(k p) b -> p k b", p=128), dx2[:])
                        for kk in range(HC):
                            nc.vector.tensor_copy(dx2[:, kk], h_st[0][(t + 1) % 2][:, kk])
                        nc.sync.dma_start(
                            dbg_d[6].rearrange("(k p) b -> p k b", p=128), dx2[:])
                        for kk in range(HC):
                            nc.vector.tensor_copy(dx2[:, kk], y_acc[:, kk])
                        nc.sync.dma_start(
                            dbg_d[7].rearrange("(k p) b -> p k b", p=128), dx2[:])

                    # ---- cross-cell mean via AllReduce ------------------
                    ar_i = dpool2.tile([H, B], F32, tag="ari", name=f"ari{t}")
                    ar_o = dpool2.tile(
                        [H, B], F32, tag="aro", name=f"aro{t}", addr_space="Shared"
                    )
                    nc.sync.dma_start(
                        ar_i.rearrange("(k p) b -> p k b", p=128), y_acc[:]
                    )
                    nc.gpsimd.collective_compute(
                        "AllReduce",
                        ALU.add,
                        ins=[ar_i.opt()],
                        outs=[ar_o.opt()],
                        replica_groups=RG,
                    )
                    if t < t_steps - 1:
                        nc.sync.dma_start(
                            ext[:], ar_o.rearrange("(k p) b -> p k b", p=128)
                        )
                    nc.sync.dma_start(out_d[t], ar_o[:])

    nc.compile()
    return nc


def prepare_inputs(tokens, emb, Wproj, bproj, Wp, bp, ln_g, ln_b,
                   Wih, bih, Whh, bhh, Wa, ba, gate_logit):
    """Host-side parameter prep + per-core sharding. Returns in_maps."""
    tokens = np.asarray(tokens).astype(np.int32)
    emb = np.asarray(emb, dtype=np.float32).copy()
    emb[0] = 0.0  # padding_idx
    emb_bf = emb.astype(BF16)

    Wproj = np.asarray(Wproj, np.float32)
    bproj = np.asarray(bproj, np.float32)
    Wp = np.asarray(Wp, np.float32)
    bp = np.asarray(bp, np.float32)
    ln_g = np.asarray(ln_g, np.float32)
    ln_b = np.asarray(ln_b, np.float32)
    Wih = np.asarray(Wih, np.float32)
    bih = np.asarray(bih, np.float32)
    Whh = np.asarray(Whh, np.float32)
    bhh = np.asarray(bhh, np.float32)
    Wa = np.asarray(Wa, np.float32)
    ba = np.asarray(ba, np.float32)
    gate_logit = np.asarray(gate_logit, np.float32)

    # Fold the LN affine (g, b) into the input-hidden weights / gate bias.
    Wih_g = Wih * ln_g[:, None, :]                       # [C, 4H, H]
    bg = bih + np.einsum("cgh,ch->cg", Wih, ln_b) + bhh  # [C, 4H]
    w1n = -Wih_g.sum(-1)                                 # [C, 4H]
    gsc = 1.0 / (1.0 + np.exp(-gate_logit)) / C          # [C]

    wproj_p = _pack_lhsT(Wproj).astype(BF16)
    bproj_p = _pack_bias(bproj[None, :])                 # [128, 4]
    ident = np.eye(128, dtype=np.float32).astype(BF16)

    in_maps = []
    for i in range(NCORES):
        cs = slice(CPC * i, CPC * (i + 1))
        wp_p = np.concatenate([_pack_lhsT(Wp[c]) for c in range(cs.start, cs.stop)], 1)
        wih_p = np.concatenate(
            [_pack_lhsT(np.ascontiguousarray(Wih_g[c].T)) for c in range(cs.start, cs.stop)], 1
        )
        whh_p = np.concatenate(
            [_pack_lhsT(np.ascontiguousarray(Whh[c].T)) for c in range(cs.start, cs.stop)], 1
        )
        wa_p = np.concatenate([_pack_lhsT(Wa[c]) for c in range(cs.start, cs.stop)], 1)

        t0 = TLOC * i
        tok_core = np.ascontiguousarray(
            tokens[:, t0:t0 + TLOC].T.reshape(NGRP * 4, 128, 1)
        )

        in_maps.append({
            "emb": emb_bf,
            "tok": tok_core,
            "wproj": wproj_p,
            "bproj": bproj_p,
            "wp": wp_p.astype(BF16),
            "wih": wih_p.astype(BF16),
            "whh": whh_p.astype(BF16),
            "wa": wa_p.astype(BF16),
            "w1n": w1n[cs].reshape(1, -1).astype(BF16),
            "bp": _pack_bias(bp[cs]),
            "bg": _pack_bias(bg[cs]),
            "ba": _pack_bias(ba[cs]),
            "gsc": np.broadcast_to(gsc[cs], (128, CPC)).astype(np.float32).copy(),
            "ident": ident,
        })
    return in_maps


_CACHE = {}


def run(inputs: dict, t_steps: int = T, trace: bool = False):
    key = t_steps
    if key not in _CACHE:
        _CACHE[key] = build_program(t_steps)
    nc = _CACHE[key]
    in_maps = prepare_inputs(**inputs)
    res = run_bass_kernel_spmd(nc, in_maps, list(range(NCORES)), trace=trace)
    ysT = res.results[0]["out"]  # [t_steps, H, B] f32
    out = np.ascontiguousarray(ysT.transpose(2, 0, 1))  # [B, t_steps, H]
    return out, res


def kernel(**inputs) -> np.ndarray:
    out, _ = run(inputs, T)
    return out


def run_timed(inputs: dict, t_steps: int = T, n_iters: int = 3):
    """Replicates bass2jax.run_bass_via_pjrt's multi-core path but keeps the
    jitted executable and device-resident inputs so repeat calls measure the
    on-device execution time (plus dispatch) rather than NEFF compile or
    host->device transfer."""
    import time
    import jax
    from jax.sharding import Mesh, PartitionSpec
    from jax.experimental.shard_map import shard_map
    from concourse import bass2jax, mybir as _mb

    key = t_steps
    if key not in _CACHE:
        _CACHE[key] = build_program(t_steps)
    nc = _CACHE[key]
    in_maps = prepare_inputs(**inputs)

    bass2jax.install_neuronx_cc_hook()
    part_name = nc.partition_id_tensor.name if nc.partition_id_tensor else None
    in_names, out_names, out_avals, zero_outs = [], [], [], []
    for alloc in nc.m.functions[0].allocations:
        if not isinstance(alloc, _mb.MemoryLocationSet):
            continue
        name = alloc.memorylocations[0].name
        if alloc.kind == "ExternalInput":
            if name != part_name:
                in_names.append(name)
        elif alloc.kind == "ExternalOutput":
            out_names.append(name)
            out_avals.append(
                jax.core.ShapedArray(alloc.tensor_shape, _mb.dt.np(alloc.dtype))
            )
            zero_outs.append(
                np.zeros(alloc.tensor_shape, dtype=_mb.dt.np(alloc.dtype))
            )
    n_params = len(in_names)
    all_names = in_names + out_names
    if part_name is not None:
        all_names.append(part_name)

    def _body(*args):
        operands = list(args)
        if part_name is not None:
            operands.append(bass2jax.partition_id_tensor())
        outs = bass2jax._bass_exec_p.bind(
            *operands,
            out_avals=tuple(out_avals),
            in_names=tuple(all_names),
            out_names=tuple(out_names),
            lowering_input_output_aliases=(),
            sim_require_finite=True,
            sim_require_nnan=True,
            nc=nc,
        )
        return tuple(outs)

    devices = jax.devices()[:NCORES]
    mesh = Mesh(np.asarray(devices), ("core",))
    n_outs = len(out_names)
    sharded = jax.jit(
        shard_map(
            _body, mesh=mesh,
            in_specs=(PartitionSpec("core"),) * (n_params + n_outs),
            out_specs=(PartitionSpec("core"),) * n_outs,
            check_rep=False,
        ),
        keep_unused=True,
    )
    concat_in = [
        np.concatenate([np.asarray(in_maps[c][nm]) for c in range(NCORES)], axis=0)
        for nm in in_names
    ]
    concat_zeros = [
        np.zeros((NCORES * z.shape[0], *z.shape[1:]), z.dtype) for z in zero_outs
    ]
    sh = jax.sharding.NamedSharding(mesh, PartitionSpec("core"))
    dev_in = [jax.device_put(a, sh) for a in concat_in]
    dev_zero = [jax.device_put(a, sh) for a in concat_zeros]
    out_arrs = sharded(*dev_in, *dev_zero)  # warm-up / compile
    jax.block_until_ready(out_arrs)
    # pipeline n_iters calls without intermediate blocking to amortize the
    # axon dispatch round-trip; calls serialize on the devices.
    n_pipe = max(n_iters, 12)
    t0 = time.perf_counter()
    rs = [sharded(*dev_in, *dev_zero) for _ in range(n_pipe)]
    jax.block_until_ready(rs)
    per_call = (time.perf_counter() - t0) / n_pipe
    idx = out_names.index("out")
    ysT = np.asarray(out_arrs[idx]).reshape(NCORES, *out_avals[idx].shape)[0]
    out = np.ascontiguousarray(ysT.transpose(2, 0, 1))
    return out, per_call
